# revision 26
# baseline (speedup 1.0000x reference)
"""MoE gate kernel (softmax + top-6 routing) for Trainium2, 8-core SPMD.

Computes, for hidden_states [16384, 4096] and gate weight [64, 4096]:
    logits = hidden_states @ weight.T
    scores = softmax(logits)
    topk_weight, topk_idx = top_k(scores, 6);  topk_weight /= sum(topk_weight)
Returns (topk_idx int32 [16384, 6], topk_weight float32 [16384, 6]).

Strategy (memory-bound regime):
  The kernel is HBM-bandwidth-bound, so x streams in fp16 (half the fp32
  bytes).  fp16 logits have ~4e-4 abs error, which can flip the top-6
  selection/order only for tokens whose adjacent top-7 logit gaps are tiny.
  Per 512-token chunk, tokens with min top-7 gap < MARGIN are compacted via a
  PE-prefix-sum + indirect-scatter into a per-chunk id list, their fp32 rows
  are gathered back from HBM, and an exact fp32 recompute produces their
  top-6 idx/weights.  The host merges the (few) fixed tokens over the fp16
  results.  Flagged fraction ~8%; everything else ships with fp16-derived
  weights (rel err ~1e-3 << 2e-2) and provably-stable indices.

Sharding: token axis split across 8 cores (2048 tokens each); weight
replicated.  Per core the hidden dim streams through the PE in 32 chunks of
128 with the (transposed) weight stationary, accumulating logits^T [64, T]
in PSUM; logits are PE-transposed back to token-major for the per-token
top-k (DVE max/max_index), and only the 6 winning logits go through exp.
"""

import sys

for _p in ("/root/.axon_site", "/root/.axon_site/_ro/trn_rl_repo",
           "/root/.axon_site/_ro/pypackages", "/opt/trn_rl_repo"):
    if _p not in sys.path:
        sys.path.append(_p)

import numpy as np

N_CORES = 8
N_TOKENS = 16384
HIDDEN = 4096
N_EXPERTS = 64
TOP_K = 6

T_CORE = N_TOKENS // N_CORES          # 2048 tokens per core
CHUNK = 512                           # tokens per pipeline chunk
N_CHUNKS = T_CORE // CHUNK            # 4
KC = HIDDEN // 128                    # 32 k-chunks of 128
KT_PER_DMA = 8                        # k-chunks per DMA (8 x 2B = 1 MiB)
N_KDMA = KC // KT_PER_DMA             # 4 DMAs per chunk
GROUPS = CHUNK // 128                 # 4 transpose groups per chunk

XBUFS = 10                            # x-tile pool depth

FIXUP = True                          # exact-fp32 fixup of ambiguous tokens
NFIX = 64                             # fixup slots per 512-token chunk
MARGIN = 2e-3                         # top-7 adjacent-gap ambiguity threshold
PROBE_PLAIN_GATHER = False            # perf probe: plain DMA instead of gather

_PROGRAM = None


def _build_program(n_iters: int = 1):
    import concourse.bacc as bacc
    import concourse.tile as tile
    import concourse.mybir as mybir
    import concourse.bass as bass
    from concourse import masks

    f32 = mybir.dt.float32
    f16 = mybir.dt.float16
    i32 = mybir.dt.int32
    u32 = mybir.dt.uint32

    nc = bacc.Bacc("TRN2", target_bir_lowering=False, debug=False,
                   num_devices=N_CORES)

    xs_h = nc.dram_tensor("xs", [N_CHUNKS, N_KDMA, 128, KT_PER_DMA, CHUNK],
                          f16, kind="ExternalInput")
    wt_h = nc.dram_tensor("wt", [128, KC, N_EXPERTS], f16,
                          kind="ExternalInput")
    oi_h = nc.dram_tensor("oidx", [T_CORE, TOP_K], i32, kind="ExternalOutput")
    ow_h = nc.dram_tensor("ow", [T_CORE, TOP_K], f32, kind="ExternalOutput")
    if FIXUP:
        wtf_h = nc.dram_tensor("wtf", [128, KC, N_EXPERTS], f32,
                               kind="ExternalInput")
        xf_h = nc.dram_tensor("xf", [T_CORE, HIDDEN], f32,
                              kind="ExternalInput")
        trio_h = nc.dram_tensor("trio", [128, 256], f16, kind="ExternalInput")
        iota_h = nc.dram_tensor("iota", [128, N_CHUNKS * GROUPS], i32,
                                kind="ExternalInput")
        padd_h = nc.dram_tensor("padd", [128, GROUPS], f32,
                                kind="ExternalInput")
        nidx_h = nc.dram_tensor("nidx", [128, NFIX], f32,
                                kind="ExternalInput")
        fids_h = [nc.dram_tensor(f"fids{c}", [NFIX, 1], i32,
                                 kind="ExternalOutput")
                  for c in range(N_CHUNKS)]
        foi_h = nc.dram_tensor("foi", [N_CHUNKS, NFIX, TOP_K], i32,
                               kind="ExternalOutput")
        fow_h = nc.dram_tensor("fow", [N_CHUNKS, NFIX, TOP_K], f32,
                               kind="ExternalOutput")
        fcnt_h = nc.dram_tensor("fcnt", [N_CHUNKS, 1], i32,
                                kind="ExternalOutput")

    with tile.TileContext(nc) as tc:
        with (
            tc.tile_pool(name="const", bufs=1) as cpool,
            tc.tile_pool(name="xin", bufs=XBUFS) as xpool,
            tc.tile_pool(name="ps_log", bufs=2, space=bass.MemorySpace.PSUM) as pslog,
            tc.tile_pool(name="ps_tr", bufs=2, space=bass.MemorySpace.PSUM) as pstr,
            tc.tile_pool(name="lg", bufs=3) as lgpool,
            tc.tile_pool(name="tk", bufs=4) as tkpool,
            tc.tile_pool(name="ps_P", bufs=1, space=bass.MemorySpace.PSUM) as psP,
            tc.tile_pool(name="ps_fl", bufs=1, space=bass.MemorySpace.PSUM) as psfl,
            tc.tile_pool(name="fl", bufs=2) as flpool,
            tc.tile_pool(name="xg", bufs=4) as xgpool,
            tc.tile_pool(name="xtf", bufs=2) as xtfpool,
            tc.tile_pool(name="fx", bufs=2) as fxpool,
        ):
            # Engine warm-up: first use of PE / ACT-exp / DVE-max pulls init
            # or ucode tables; issue them on dummies so the loads overlap.
            wrm = cpool.tile([128, 16], f32)
            nc.gpsimd.memset(wrm[:], 0.0)
            wrm_ps = pslog.tile([N_EXPERTS, CHUNK], f32, tag="log")
            nc.tensor.matmul(wrm_ps[0:16, 0:16], wrm[:, 0:16], wrm[:])
            wrm_e = cpool.tile([128, 16], f32)
            nc.scalar.activation(wrm_e[:], wrm[:],
                                 mybir.ActivationFunctionType.Exp)
            wrm_m = cpool.tile([128, 8], f32)
            nc.vector.max(wrm_m[:], wrm[:])
            wrm_i = cpool.tile([128, 8], u32)
            nc.vector.max_index(wrm_i[:], wrm_m[:], wrm[:])

            wt_sb = cpool.tile([128, KC, N_EXPERTS], f16)
            nc.gpsimd.dma_start(wt_sb[:], wt_h.ap())
            ident = cpool.tile([64, 64], f32)
            masks.make_identity(nc, ident[:])
            if FIXUP:
                wtf_sb = cpool.tile([128, KC, N_EXPERTS], f32)
                nc.gpsimd.dma_start(wtf_sb[:], wtf_h.ap())
                trio = cpool.tile([128, 256], f16)
                nc.gpsimd.dma_start(trio[:], trio_h.ap())
                iota = cpool.tile([128, N_CHUNKS * GROUPS], i32)
                nc.gpsimd.dma_start(iota[:], iota_h.ap())
                padd = cpool.tile([128, GROUPS], f32)
                nc.gpsimd.dma_start(padd[:], padd_h.ap())
                nidx = cpool.tile([128, NFIX], f32)
                nc.gpsimd.dma_start(nidx[:], nidx_h.ap())
                iof = cpool.tile([128, N_CHUNKS * GROUPS], f32)
                nc.vector.tensor_copy(iof[:], iota[:])

            def phase_b(xg, c):
                    # fixup compute for chunk c (runs one chunk delayed so the
                    # PE never stalls on the gather DMA)
                    xtf = xtfpool.tile([128, KC, NFIX], f32)
                    for k in range(KC):
                        ps_fx = pstr.tile([128, GROUPS, N_EXPERTS], f32, tag="tr")
                        nc.tensor.transpose(ps_fx[:, 0, :],
                                            xg[:, k * 128:(k + 1) * 128],
                                            ident[:])
                        nc.vector.tensor_copy(xtf[:, k, :], ps_fx[:, 0, :])
                    ps_fl = psfl.tile([N_EXPERTS, NFIX], f32)
                    for k in range(KC):
                        nc.tensor.matmul(ps_fl[:], wtf_sb[:, k, :],
                                         xtf[:, k, :],
                                         start=(k == 0), stop=(k == KC - 1))
                    tlf = fxpool.tile([N_EXPERTS, NFIX], f32)
                    nc.scalar.activation(tlf[:], ps_fl[:],
                                         mybir.ActivationFunctionType.Copy)
                    ps_ftt = pstr.tile([128, GROUPS, N_EXPERTS], f32, tag="tr")
                    ps_ft = ps_ftt[0:NFIX, 0, :]
                    nc.tensor.transpose(ps_ft, tlf[:], ident[:])
                    scf = fxpool.tile([NFIX, N_EXPERTS], f32)
                    nc.vector.tensor_copy(scf[:], ps_ft)
                    # exact top-6 + softmax weights for the fixed tokens
                    l8f = fxpool.tile([NFIX, 8], f32)
                    ix8f = fxpool.tile([NFIX, 8], u32)
                    nc.vector.max(l8f[:], scf[:])
                    nc.vector.max_index(ix8f[:], l8f[:], scf[:])
                    i6f = fxpool.tile([NFIX, TOP_K], i32)
                    nc.vector.tensor_copy(i6f[:], ix8f[:, 0:TOP_K])
                    nc.sync.dma_start(foi_h.ap()[c], i6f[:])
                    negmf = fxpool.tile([NFIX, 1], f32)
                    nc.vector.tensor_scalar_mul(negmf[:], l8f[:, 0:1], -1.0)
                    e6f = fxpool.tile([NFIX, TOP_K], f32)
                    nc.scalar.activation(e6f[:], l8f[:, 0:TOP_K],
                                         mybir.ActivationFunctionType.Exp,
                                         bias=negmf[:])
                    denf = fxpool.tile([NFIX, 1], f32)
                    nc.vector.reduce_sum(denf[:], e6f[:],
                                         axis=mybir.AxisListType.X)
                    recf = fxpool.tile([NFIX, 1], f32)
                    nc.vector.reciprocal(recf[:], denf[:])
                    w6f = fxpool.tile([NFIX, TOP_K], f32)
                    nc.vector.tensor_scalar(w6f[:], e6f[:], recf[:], None,
                                            mybir.AluOpType.mult)
                    nc.scalar.dma_start(fow_h.ap()[c], w6f[:])

            for it in range(n_iters):
                pending = []
                for c in range(N_CHUNKS):
                    ps_l = pslog.tile([N_EXPERTS, CHUNK], f32, tag="log")
                    for j in range(N_KDMA):
                        xt = xpool.tile([128, KT_PER_DMA, CHUNK], f16)
                        eng = nc.sync if (c * N_KDMA + j) % 2 == 0 else nc.scalar
                        eng.dma_start(xt[:], xs_h.ap()[c, j])
                        for kt in range(KT_PER_DMA):
                            k = j * KT_PER_DMA + kt
                            nc.tensor.matmul(
                                ps_l[:], wt_sb[:, k, :], xt[:, kt, :],
                                start=(k == 0), stop=(k == KC - 1),
                            )
                    # logits^T [64, CHUNK] -> sbuf
                    tl = lgpool.tile([N_EXPERTS, CHUNK], f32)
                    nc.scalar.activation(tl[:], ps_l[:],
                                         mybir.ActivationFunctionType.Copy)
                    # transpose to token-major [128, 64] per 128-token group
                    ps_t = pstr.tile([128, GROUPS, N_EXPERTS], f32, tag="tr")
                    sc = lgpool.tile([128, GROUPS, N_EXPERTS], f32)
                    for g in range(GROUPS):
                        nc.tensor.transpose(ps_t[:, g, :],
                                            tl[:, g * 128:(g + 1) * 128],
                                            ident[:])
                        nc.vector.tensor_copy(sc[:, g, :], ps_t[:, g, :])
                    # top-8 values + indices per token (on fp16-path logits)
                    l8 = tkpool.tile([128, GROUPS, 8], f32)
                    ix8 = tkpool.tile([128, GROUPS, 8], u32)
                    for g in range(GROUPS):
                        nc.vector.max(l8[:, g, :], sc[:, g, :])
                        nc.vector.max_index(ix8[:, g, :], l8[:, g, :],
                                            sc[:, g, :])
                    off = c * CHUNK * TOP_K
                    pat = [[TOP_K, 128], [128 * TOP_K, GROUPS], [1, TOP_K]]
                    i6 = tkpool.tile([128, GROUPS, TOP_K], i32)
                    nc.vector.tensor_copy(i6[:], ix8[:, :, 0:TOP_K])
                    nc.sync.dma_start(bass.AP(oi_h, off, pat), i6[:])
                    # weights: exp(l_j - l_max) of the 6 winners, normalized.
                    negm = tkpool.tile([128, GROUPS], f32)
                    nc.vector.tensor_scalar_mul(negm[:], l8[:, :, 0], -1.0)
                    e6 = tkpool.tile([128, GROUPS, TOP_K], f32)
                    for g in range(GROUPS):
                        nc.scalar.activation(e6[:, g, :], l8[:, g, 0:TOP_K],
                                             mybir.ActivationFunctionType.Exp,
                                             bias=negm[:, g:g + 1])
                    den = tkpool.tile([128, GROUPS], f32)
                    nc.vector.reduce_sum(den[:], e6[:],
                                         axis=mybir.AxisListType.X)
                    rec = tkpool.tile([128, GROUPS], f32)
                    nc.vector.reciprocal(rec[:], den[:])
                    w6 = tkpool.tile([128, GROUPS, TOP_K], f32)
                    nc.vector.tensor_mul(
                        w6[:], e6[:],
                        rec[:].unsqueeze(2).broadcast_to((128, GROUPS, TOP_K)))
                    nc.scalar.dma_start(bass.AP(ow_h, off, pat), w6[:])

                    if not FIXUP:
                        continue
                    # ---- ambiguity flags: min adjacent gap in top-7 < MARGIN
                    gaps = flpool.tile([128, GROUPS, 7], f32)
                    nc.vector.tensor_sub(gaps[:], l8[:, :, 0:7], l8[:, :, 1:8])
                    ming = flpool.tile([128, GROUPS], f32)
                    nc.vector.tensor_reduce(ming[:], gaps[:],
                                            axis=mybir.AxisListType.X,
                                            op=mybir.AluOpType.min)
                    flagf = flpool.tile([128, GROUPS], f32)
                    nc.vector.tensor_scalar(flagf[:], ming[:], MARGIN, None,
                                            mybir.AluOpType.is_lt)
                    flag16 = flpool.tile([128, GROUPS], f16)
                    nc.vector.tensor_copy(flag16[:], flagf[:])
                    flagi = flpool.tile([128, GROUPS], i32)
                    nc.vector.tensor_copy(flagi[:], flagf[:])
                    # exclusive cumsum of per-column sums (free dim)
                    ecs = flpool.tile([128, GROUPS], f16)
                    nc.vector.memset(ecs[:, 0:1], 0.0)
                    for g in range(1, GROUPS):
                        nc.vector.tensor_add(ecs[:, g:g + 1], ecs[:, g - 1:g],
                                             flag16[:, g - 1:g])
                    # exclusive prefix over the chunk: tri-prefix within
                    # column + all-partition sums of previous columns
                    ps_PS = psP.tile([128, 2 * GROUPS], f32)
                    ps_P = ps_PS[:, 0:GROUPS]
                    nc.tensor.matmul(ps_P, trio[:, 0:128], flag16[:],
                                     start=True, stop=False)
                    nc.tensor.matmul(ps_P, trio[:, 128:256], ecs[:],
                                     start=False, stop=True)
                    # per-column totals -> chunk flag count (partition 0)
                    ps_S = ps_PS[0:1, GROUPS:2 * GROUPS]
                    nc.tensor.matmul(ps_S, trio[:, 128:129], flag16[:])
                    cntf = flpool.tile([1, 1], f32)
                    nc.vector.reduce_sum(cntf[:], ps_S,
                                         axis=mybir.AxisListType.X)
                    cnti = flpool.tile([1, 1], i32)
                    nc.vector.tensor_copy(cnti[:], cntf[:])
                    nc.sync.dma_start(fcnt_h.ap()[c], cnti[:])
                    # dest slot: flagged -> prefix P, else NFIX + local id
                    dest_f = flpool.tile([128, GROUPS], f32)
                    nc.vector.select(dest_f[:], flagi[:], ps_P[:], padd[:])
                    # compact on the PE: onehot M[t, n] = (dest[t] == n),
                    # ids[n] = sum_t M[t, n] * token_id[t]
                    onehot = flpool.tile([128, GROUPS, NFIX], f32)
                    for g in range(GROUPS):
                        nc.vector.tensor_tensor(
                            out=onehot[:, g, :],
                            in0=dest_f[:, g:g + 1].broadcast_to((128, NFIX)),
                            in1=nidx[:],
                            op=mybir.AluOpType.is_equal)
                    ps_ids = psP.tile([NFIX, 1], f32, tag="ids")
                    for g in range(GROUPS):
                        ga = c * GROUPS + g
                        nc.tensor.matmul(ps_ids[:], onehot[:, g, :],
                                         iof[:, ga:ga + 1],
                                         start=(g == 0), stop=(g == GROUPS - 1))
                    ids_cl = fxpool.tile([NFIX, 1], i32)
                    nc.vector.tensor_copy(ids_cl[:], ps_ids[:])
                    nc.sync.dma_start(fids_h[c].ap(), ids_cl[:])
                    # gather the flagged tokens' fp32 rows
                    xg = xgpool.tile([NFIX, HIDDEN], f32)
                    if PROBE_PLAIN_GATHER:
                        nc.gpsimd.dma_start(xg[:], xf_h.ap()[c * NFIX:(c + 1) * NFIX])
                    else:
                        nc.gpsimd.indirect_dma_start(
                            out=xg[:],
                            out_offset=None,
                            in_=xf_h.ap(),
                            in_offset=bass.IndirectOffsetOnAxis(ap=ids_cl[:],
                                                                axis=0),
                        )
                    pending.append((xg, c))
                    if len(pending) > 3:
                        phase_b(*pending.pop(0))
                for xg, c in pending:
                    phase_b(xg, c)

    nc.compile()
    return nc


def _get_program():
    global _PROGRAM
    if _PROGRAM is None:
        _PROGRAM = _build_program(1)
    return _PROGRAM


def _prep_inputs(hidden_states: np.ndarray, weight: np.ndarray):
    """Build per-core input maps (token-sharded x, replicated weight)."""
    w = np.ascontiguousarray(weight.astype(np.float32, copy=False))
    # wt[p, k, e] = W[e, k*128 + p]
    wtf = np.ascontiguousarray(w.T.reshape(KC, 128, N_EXPERTS)
                               .transpose(1, 0, 2))
    wt16 = wtf.astype(np.float16)
    if FIXUP:
        tri = np.tril(np.ones((128, 128), np.float32), -1).T
        trio = np.ascontiguousarray(
            np.concatenate([tri, np.ones((128, 128), np.float32)],
                           axis=1)).astype(np.float16)
        iota = np.ascontiguousarray(
            (np.arange(N_CHUNKS * GROUPS)[None, :] * 128 +
             np.arange(128)[:, None])).astype(np.int32)
        padd = np.ascontiguousarray(
            NFIX + np.arange(GROUPS)[None, :] * 128 +
            np.arange(128)[:, None]).astype(np.float32)
        nidx = np.ascontiguousarray(
            np.broadcast_to(np.arange(NFIX, dtype=np.float32)[None, :],
                            (128, NFIX)))
    in_maps = []
    for cid in range(N_CORES):
        shard = np.ascontiguousarray(
            hidden_states[cid * T_CORE:(cid + 1) * T_CORE]
            .astype(np.float32, copy=False))
        # xs[c, j, p, kt, t] = shard[c*512 + t, (j*KT + kt)*128 + p]
        xs = (shard.astype(np.float16).T
              .reshape(N_KDMA, KT_PER_DMA, 128, N_CHUNKS, CHUNK)
              .transpose(3, 0, 2, 1, 4))
        m = {"xs": np.ascontiguousarray(xs), "wt": wt16}
        if FIXUP:
            m.update({"wtf": wtf, "xf": shard, "trio": trio, "iota": iota,
                      "padd": padd, "nidx": nidx})
        in_maps.append(m)
    return in_maps


def kernel(hidden_states: np.ndarray, weight: np.ndarray):
    from concourse.bass_utils import run_bass_kernel_spmd

    hidden_states = np.asarray(hidden_states)
    weight = np.asarray(weight)
    nc = _get_program()
    in_maps = _prep_inputs(hidden_states, weight)
    res = run_bass_kernel_spmd(nc, in_maps, list(range(N_CORES)),
                               trace=False)
    idx_parts, wgt_parts = [], []
    for i in range(N_CORES):
        r = res.results[i]
        idx = np.array(r["oidx"], dtype=np.int32, copy=True)
        wgt = np.array(r["ow"], dtype=np.float32, copy=True)
        if FIXUP:
            for c in range(N_CHUNKS):
                n = min(int(r["fcnt"][c, 0]), NFIX)
                if n <= 0:
                    continue
                ids = r[f"fids{c}"][:n, 0]
                valid = (ids >= 0) & (ids < T_CORE)
                ids = ids[valid]
                idx[ids] = r["foi"][c, :n][valid]
                wgt[ids] = r["fow"][c, :n][valid]
        idx_parts.append(idx)
        wgt_parts.append(wgt)
    return (np.concatenate(idx_parts, axis=0),
            np.concatenate(wgt_parts, axis=0))


# revision 27
# speedup vs baseline: 1.1282x; 1.1282x over previous
"""MoE gate kernel (softmax + top-6 routing) for Trainium2, 8-core SPMD.

Computes, for hidden_states [16384, 4096] and gate weight [64, 4096]:
    logits = hidden_states @ weight.T
    scores = softmax(logits)
    topk_weight, topk_idx = top_k(scores, 6);  topk_weight /= sum(topk_weight)
Returns (topk_idx int32 [16384, 6], topk_weight float32 [16384, 6]).

Strategy (memory-bound regime):
  The kernel is HBM-bandwidth-bound, so x streams in fp16 (half the fp32
  bytes).  fp16 logits have ~4e-4 abs error, which can flip the top-6
  selection/order only for tokens whose adjacent top-7 logit gaps are tiny.
  Per 512-token chunk, tokens with min top-7 gap < MARGIN are compacted via a
  PE-prefix-sum + indirect-scatter into a per-chunk id list, their fp32 rows
  are gathered back from HBM, and an exact fp32 recompute produces their
  top-6 idx/weights.  The host merges the (few) fixed tokens over the fp16
  results.  Flagged fraction ~8%; everything else ships with fp16-derived
  weights (rel err ~1e-3 << 2e-2) and provably-stable indices.

Sharding: token axis split across 8 cores (2048 tokens each); weight
replicated.  Per core the hidden dim streams through the PE in 32 chunks of
128 with the (transposed) weight stationary, accumulating logits^T [64, T]
in PSUM; logits are PE-transposed back to token-major for the per-token
top-k (DVE max/max_index), and only the 6 winning logits go through exp.
"""

import sys

for _p in ("/root/.axon_site", "/root/.axon_site/_ro/trn_rl_repo",
           "/root/.axon_site/_ro/pypackages", "/opt/trn_rl_repo"):
    if _p not in sys.path:
        sys.path.append(_p)

import numpy as np

N_CORES = 8
N_TOKENS = 16384
HIDDEN = 4096
N_EXPERTS = 64
TOP_K = 6

T_CORE = N_TOKENS // N_CORES          # 2048 tokens per core
CHUNK = 512                           # tokens per pipeline chunk
N_CHUNKS = T_CORE // CHUNK            # 4
KC = HIDDEN // 128                    # 32 k-chunks of 128
KT_PER_DMA = 8                        # k-chunks per DMA (8 x 2B = 1 MiB)
N_KDMA = KC // KT_PER_DMA             # 4 DMAs per chunk
GROUPS = CHUNK // 128                 # 4 transpose groups per chunk

XBUFS = 12                            # x-tile pool depth

FIXUP = True                          # exact-fp32 fixup of ambiguous tokens
NFIX = 64                             # fixup slots per 512-token chunk
MARGIN = 2e-3                         # top-7 adjacent-gap ambiguity threshold
PROBE_PLAIN_GATHER = False            # perf probe: plain DMA instead of gather

_PROGRAM = None


def _build_program(n_iters: int = 1):
    import concourse.bacc as bacc
    import concourse.tile as tile
    import concourse.mybir as mybir
    import concourse.bass as bass
    from concourse import masks

    f32 = mybir.dt.float32
    f16 = mybir.dt.float16
    i32 = mybir.dt.int32
    u32 = mybir.dt.uint32

    nc = bacc.Bacc("TRN2", target_bir_lowering=False, debug=False,
                   num_devices=N_CORES)

    xs_h = nc.dram_tensor("xs", [N_CHUNKS, N_KDMA, 128, KT_PER_DMA, CHUNK],
                          f16, kind="ExternalInput")
    wt_h = nc.dram_tensor("wt", [128, KC, N_EXPERTS], f16,
                          kind="ExternalInput")
    oi_h = nc.dram_tensor("oidx", [T_CORE, TOP_K], i32, kind="ExternalOutput")
    ow_h = nc.dram_tensor("ow", [T_CORE, TOP_K], f32, kind="ExternalOutput")
    if FIXUP:
        wtf_h = nc.dram_tensor("wtf", [128, KC, N_EXPERTS], f32,
                               kind="ExternalInput")
        xf_h = nc.dram_tensor("xf", [T_CORE, HIDDEN], f32,
                              kind="ExternalInput")
        trio_h = nc.dram_tensor("trio", [128, 256], f16, kind="ExternalInput")
        iota_h = nc.dram_tensor("iota", [128, N_CHUNKS * GROUPS], i32,
                                kind="ExternalInput")
        padd_h = nc.dram_tensor("padd", [128, GROUPS], f32,
                                kind="ExternalInput")
        nidx_h = nc.dram_tensor("nidx", [128, NFIX], f32,
                                kind="ExternalInput")
        fids_h = [nc.dram_tensor(f"fids{c}", [NFIX, 1], i32,
                                 kind="ExternalOutput")
                  for c in range(N_CHUNKS)]
        foi_h = nc.dram_tensor("foi", [N_CHUNKS, NFIX, TOP_K], i32,
                               kind="ExternalOutput")
        fow_h = nc.dram_tensor("fow", [N_CHUNKS, NFIX, TOP_K], f32,
                               kind="ExternalOutput")
        fcnt_h = nc.dram_tensor("fcnt", [N_CHUNKS, 1], i32,
                                kind="ExternalOutput")

    with tile.TileContext(nc) as tc:
        with (
            tc.tile_pool(name="const", bufs=1) as cpool,
            tc.tile_pool(name="xin", bufs=XBUFS) as xpool,
            tc.tile_pool(name="ps_log", bufs=2, space=bass.MemorySpace.PSUM) as pslog,
            tc.tile_pool(name="ps_tr", bufs=2, space=bass.MemorySpace.PSUM) as pstr,
            tc.tile_pool(name="lg", bufs=3) as lgpool,
            tc.tile_pool(name="tk", bufs=4) as tkpool,
            tc.tile_pool(name="ps_P", bufs=1, space=bass.MemorySpace.PSUM) as psP,
            tc.tile_pool(name="ps_fl", bufs=1, space=bass.MemorySpace.PSUM) as psfl,
            tc.tile_pool(name="fl", bufs=2) as flpool,
            tc.tile_pool(name="xg", bufs=3) as xgpool,
            tc.tile_pool(name="xtf", bufs=2) as xtfpool,
            tc.tile_pool(name="fx", bufs=2) as fxpool,
        ):
            # Engine warm-up: first use of PE / ACT-exp / DVE-max pulls init
            # or ucode tables; issue them on dummies so the loads overlap.
            wrm = cpool.tile([128, 16], f32)
            nc.gpsimd.memset(wrm[:], 0.0)
            wrm_ps = pslog.tile([N_EXPERTS, CHUNK], f32, tag="log")
            nc.tensor.matmul(wrm_ps[0:16, 0:16], wrm[:, 0:16], wrm[:])
            wrm_e = cpool.tile([128, 16], f32)
            nc.scalar.activation(wrm_e[:], wrm[:],
                                 mybir.ActivationFunctionType.Exp)
            wrm_m = cpool.tile([128, 8], f32)
            nc.vector.max(wrm_m[:], wrm[:])
            wrm_i = cpool.tile([128, 8], u32)
            nc.vector.max_index(wrm_i[:], wrm_m[:], wrm[:])

            wt_sb = cpool.tile([128, KC, N_EXPERTS], f16)
            nc.gpsimd.dma_start(wt_sb[:], wt_h.ap())
            ident = cpool.tile([64, 64], f32)
            masks.make_identity(nc, ident[:])
            if FIXUP:
                wtf_sb = cpool.tile([128, KC, N_EXPERTS], f32)
                nc.gpsimd.dma_start(wtf_sb[:], wtf_h.ap())
                trio = cpool.tile([128, 256], f16)
                nc.gpsimd.dma_start(trio[:], trio_h.ap())
                iota = cpool.tile([128, N_CHUNKS * GROUPS], i32)
                nc.gpsimd.dma_start(iota[:], iota_h.ap())
                padd = cpool.tile([128, GROUPS], f32)
                nc.gpsimd.dma_start(padd[:], padd_h.ap())
                nidx = cpool.tile([128, NFIX], f32)
                nc.gpsimd.dma_start(nidx[:], nidx_h.ap())
                iof = cpool.tile([128, N_CHUNKS * GROUPS], f32)
                nc.vector.tensor_copy(iof[:], iota[:])

            def phase_b(xg, c):
                    # fixup compute for chunk c (runs one chunk delayed so the
                    # PE never stalls on the gather DMA)
                    xtf = xtfpool.tile([128, KC, NFIX], f32)
                    for k in range(KC):
                        ps_fx = pstr.tile([128, GROUPS, N_EXPERTS], f32, tag="tr")
                        nc.tensor.transpose(ps_fx[:, 0, :],
                                            xg[:, k * 128:(k + 1) * 128],
                                            ident[:])
                        nc.vector.tensor_copy(xtf[:, k, :], ps_fx[:, 0, :])
                    ps_fl = psfl.tile([N_EXPERTS, NFIX], f32)
                    for k in range(KC):
                        nc.tensor.matmul(ps_fl[:], wtf_sb[:, k, :],
                                         xtf[:, k, :],
                                         start=(k == 0), stop=(k == KC - 1))
                    tlf = fxpool.tile([N_EXPERTS, NFIX], f32)
                    nc.scalar.activation(tlf[:], ps_fl[:],
                                         mybir.ActivationFunctionType.Copy)
                    ps_ftt = pstr.tile([128, GROUPS, N_EXPERTS], f32, tag="tr")
                    ps_ft = ps_ftt[0:NFIX, 0, :]
                    nc.tensor.transpose(ps_ft, tlf[:], ident[:])
                    scf = fxpool.tile([NFIX, N_EXPERTS], f32)
                    nc.vector.tensor_copy(scf[:], ps_ft)
                    # exact top-6 + softmax weights for the fixed tokens
                    l8f = fxpool.tile([NFIX, 8], f32)
                    ix8f = fxpool.tile([NFIX, 8], u32)
                    nc.vector.max(l8f[:], scf[:])
                    nc.vector.max_index(ix8f[:], l8f[:], scf[:])
                    i6f = fxpool.tile([NFIX, TOP_K], i32)
                    nc.vector.tensor_copy(i6f[:], ix8f[:, 0:TOP_K])
                    nc.sync.dma_start(foi_h.ap()[c], i6f[:])
                    negmf = fxpool.tile([NFIX, 1], f32)
                    nc.vector.tensor_scalar_mul(negmf[:], l8f[:, 0:1], -1.0)
                    e6f = fxpool.tile([NFIX, TOP_K], f32)
                    nc.scalar.activation(e6f[:], l8f[:, 0:TOP_K],
                                         mybir.ActivationFunctionType.Exp,
                                         bias=negmf[:])
                    denf = fxpool.tile([NFIX, 1], f32)
                    nc.vector.reduce_sum(denf[:], e6f[:],
                                         axis=mybir.AxisListType.X)
                    recf = fxpool.tile([NFIX, 1], f32)
                    nc.vector.reciprocal(recf[:], denf[:])
                    w6f = fxpool.tile([NFIX, TOP_K], f32)
                    nc.vector.tensor_scalar(w6f[:], e6f[:], recf[:], None,
                                            mybir.AluOpType.mult)
                    nc.scalar.dma_start(fow_h.ap()[c], w6f[:])

            for it in range(n_iters):
                pending = []
                for c in range(N_CHUNKS):
                    ps_l = pslog.tile([N_EXPERTS, CHUNK], f32, tag="log")
                    for j in range(N_KDMA):
                        xt = xpool.tile([128, KT_PER_DMA, CHUNK], f16)
                        eng = nc.sync if (c * N_KDMA + j) % 2 == 0 else nc.scalar
                        eng.dma_start(xt[:], xs_h.ap()[c, j])
                        for kt in range(KT_PER_DMA):
                            k = j * KT_PER_DMA + kt
                            nc.tensor.matmul(
                                ps_l[:], wt_sb[:, k, :], xt[:, kt, :],
                                start=(k == 0), stop=(k == KC - 1),
                            )
                    # logits^T [64, CHUNK] -> sbuf
                    tl = lgpool.tile([N_EXPERTS, CHUNK], f32)
                    nc.scalar.activation(tl[:], ps_l[:],
                                         mybir.ActivationFunctionType.Copy)
                    # transpose to token-major [128, 64] per 128-token group
                    ps_t = pstr.tile([128, GROUPS, N_EXPERTS], f32, tag="tr")
                    sc = lgpool.tile([128, GROUPS, N_EXPERTS], f32)
                    for g in range(GROUPS):
                        nc.tensor.transpose(ps_t[:, g, :],
                                            tl[:, g * 128:(g + 1) * 128],
                                            ident[:])
                        nc.vector.tensor_copy(sc[:, g, :], ps_t[:, g, :])
                    # top-8 values + indices per token (on fp16-path logits)
                    l8 = tkpool.tile([128, GROUPS, 8], f32)
                    ix8 = tkpool.tile([128, GROUPS, 8], u32)
                    for g in range(GROUPS):
                        nc.vector.max(l8[:, g, :], sc[:, g, :])
                        nc.vector.max_index(ix8[:, g, :], l8[:, g, :],
                                            sc[:, g, :])
                    off = c * CHUNK * TOP_K
                    pat = [[TOP_K, 128], [128 * TOP_K, GROUPS], [1, TOP_K]]
                    i6 = tkpool.tile([128, GROUPS, TOP_K], i32)
                    nc.vector.tensor_copy(i6[:], ix8[:, :, 0:TOP_K])
                    nc.sync.dma_start(bass.AP(oi_h, off, pat), i6[:])
                    # weights: exp(l_j - l_max) of the 6 winners, normalized.
                    negm = tkpool.tile([128, GROUPS], f32)
                    nc.vector.tensor_scalar_mul(negm[:], l8[:, :, 0], -1.0)
                    e6 = tkpool.tile([128, GROUPS, TOP_K], f32)
                    for g in range(GROUPS):
                        nc.scalar.activation(e6[:, g, :], l8[:, g, 0:TOP_K],
                                             mybir.ActivationFunctionType.Exp,
                                             bias=negm[:, g:g + 1])
                    den = tkpool.tile([128, GROUPS], f32)
                    nc.vector.reduce_sum(den[:], e6[:],
                                         axis=mybir.AxisListType.X)
                    rec = tkpool.tile([128, GROUPS], f32)
                    nc.vector.reciprocal(rec[:], den[:])
                    w6 = tkpool.tile([128, GROUPS, TOP_K], f32)
                    nc.vector.tensor_mul(
                        w6[:], e6[:],
                        rec[:].unsqueeze(2).broadcast_to((128, GROUPS, TOP_K)))
                    nc.scalar.dma_start(bass.AP(ow_h, off, pat), w6[:])

                    if not FIXUP:
                        continue
                    # ---- ambiguity flags: min adjacent gap in top-7 < MARGIN
                    gaps = flpool.tile([128, GROUPS, 7], f32)
                    nc.vector.tensor_sub(gaps[:], l8[:, :, 0:7], l8[:, :, 1:8])
                    ming = flpool.tile([128, GROUPS], f32)
                    nc.vector.tensor_reduce(ming[:], gaps[:],
                                            axis=mybir.AxisListType.X,
                                            op=mybir.AluOpType.min)
                    flagf = flpool.tile([128, GROUPS], f32)
                    nc.vector.tensor_scalar(flagf[:], ming[:], MARGIN, None,
                                            mybir.AluOpType.is_lt)
                    flag16 = flpool.tile([128, GROUPS], f16)
                    nc.vector.tensor_copy(flag16[:], flagf[:])
                    flagi = flpool.tile([128, GROUPS], i32)
                    nc.vector.tensor_copy(flagi[:], flagf[:])
                    # exclusive cumsum of per-column sums (free dim)
                    ecs = flpool.tile([128, GROUPS], f16)
                    nc.vector.memset(ecs[:, 0:1], 0.0)
                    for g in range(1, GROUPS):
                        nc.vector.tensor_add(ecs[:, g:g + 1], ecs[:, g - 1:g],
                                             flag16[:, g - 1:g])
                    # exclusive prefix over the chunk: tri-prefix within
                    # column + all-partition sums of previous columns
                    ps_PS = psP.tile([128, 2 * GROUPS], f32)
                    ps_P = ps_PS[:, 0:GROUPS]
                    nc.tensor.matmul(ps_P, trio[:, 0:128], flag16[:],
                                     start=True, stop=False)
                    nc.tensor.matmul(ps_P, trio[:, 128:256], ecs[:],
                                     start=False, stop=True)
                    # per-column totals -> chunk flag count (partition 0)
                    ps_S = ps_PS[0:1, GROUPS:2 * GROUPS]
                    nc.tensor.matmul(ps_S, trio[:, 128:129], flag16[:])
                    cntf = flpool.tile([1, 1], f32)
                    nc.vector.reduce_sum(cntf[:], ps_S,
                                         axis=mybir.AxisListType.X)
                    cnti = flpool.tile([1, 1], i32)
                    nc.vector.tensor_copy(cnti[:], cntf[:])
                    nc.sync.dma_start(fcnt_h.ap()[c], cnti[:])
                    # dest slot: flagged -> prefix P, else NFIX + local id
                    dest_f = flpool.tile([128, GROUPS], f32)
                    nc.vector.select(dest_f[:], flagi[:], ps_P[:], padd[:])
                    # compact on the PE: onehot M[t, n] = (dest[t] == n),
                    # ids[n] = sum_t M[t, n] * token_id[t]
                    onehot = flpool.tile([128, GROUPS, NFIX], f32)
                    for g in range(GROUPS):
                        nc.vector.tensor_tensor(
                            out=onehot[:, g, :],
                            in0=dest_f[:, g:g + 1].broadcast_to((128, NFIX)),
                            in1=nidx[:],
                            op=mybir.AluOpType.is_equal)
                    ps_ids = psP.tile([NFIX, 1], f32, tag="ids")
                    for g in range(GROUPS):
                        ga = c * GROUPS + g
                        nc.tensor.matmul(ps_ids[:], onehot[:, g, :],
                                         iof[:, ga:ga + 1],
                                         start=(g == 0), stop=(g == GROUPS - 1))
                    ids_cl = fxpool.tile([NFIX, 1], i32)
                    nc.vector.tensor_copy(ids_cl[:], ps_ids[:])
                    nc.sync.dma_start(fids_h[c].ap(), ids_cl[:])
                    # gather the flagged tokens' fp32 rows
                    xg = xgpool.tile([NFIX, HIDDEN], f32)
                    if PROBE_PLAIN_GATHER:
                        nc.gpsimd.dma_start(xg[:], xf_h.ap()[c * NFIX:(c + 1) * NFIX])
                    else:
                        nc.gpsimd.indirect_dma_start(
                            out=xg[:],
                            out_offset=None,
                            in_=xf_h.ap(),
                            in_offset=bass.IndirectOffsetOnAxis(ap=ids_cl[:],
                                                                axis=0),
                        )
                    pending.append((xg, c))
                    if len(pending) > 2:
                        phase_b(*pending.pop(0))
                for xg, c in pending:
                    phase_b(xg, c)

    nc.compile()
    return nc


def _get_program():
    global _PROGRAM
    if _PROGRAM is None:
        _PROGRAM = _build_program(1)
    return _PROGRAM


def _prep_inputs(hidden_states: np.ndarray, weight: np.ndarray):
    """Build per-core input maps (token-sharded x, replicated weight)."""
    w = np.ascontiguousarray(weight.astype(np.float32, copy=False))
    # wt[p, k, e] = W[e, k*128 + p]
    wtf = np.ascontiguousarray(w.T.reshape(KC, 128, N_EXPERTS)
                               .transpose(1, 0, 2))
    wt16 = wtf.astype(np.float16)
    if FIXUP:
        tri = np.tril(np.ones((128, 128), np.float32), -1).T
        trio = np.ascontiguousarray(
            np.concatenate([tri, np.ones((128, 128), np.float32)],
                           axis=1)).astype(np.float16)
        iota = np.ascontiguousarray(
            (np.arange(N_CHUNKS * GROUPS)[None, :] * 128 +
             np.arange(128)[:, None])).astype(np.int32)
        padd = np.ascontiguousarray(
            NFIX + np.arange(GROUPS)[None, :] * 128 +
            np.arange(128)[:, None]).astype(np.float32)
        nidx = np.ascontiguousarray(
            np.broadcast_to(np.arange(NFIX, dtype=np.float32)[None, :],
                            (128, NFIX)))
    in_maps = []
    for cid in range(N_CORES):
        shard = np.ascontiguousarray(
            hidden_states[cid * T_CORE:(cid + 1) * T_CORE]
            .astype(np.float32, copy=False))
        # xs[c, j, p, kt, t] = shard[c*512 + t, (j*KT + kt)*128 + p]
        xs = (shard.astype(np.float16).T
              .reshape(N_KDMA, KT_PER_DMA, 128, N_CHUNKS, CHUNK)
              .transpose(3, 0, 2, 1, 4))
        m = {"xs": np.ascontiguousarray(xs), "wt": wt16}
        if FIXUP:
            m.update({"wtf": wtf, "xf": shard, "trio": trio, "iota": iota,
                      "padd": padd, "nidx": nidx})
        in_maps.append(m)
    return in_maps


def kernel(hidden_states: np.ndarray, weight: np.ndarray):
    from concourse.bass_utils import run_bass_kernel_spmd

    hidden_states = np.asarray(hidden_states)
    weight = np.asarray(weight)
    nc = _get_program()
    in_maps = _prep_inputs(hidden_states, weight)
    res = run_bass_kernel_spmd(nc, in_maps, list(range(N_CORES)),
                               trace=False)
    idx_parts, wgt_parts = [], []
    for i in range(N_CORES):
        r = res.results[i]
        idx = np.array(r["oidx"], dtype=np.int32, copy=True)
        wgt = np.array(r["ow"], dtype=np.float32, copy=True)
        if FIXUP:
            for c in range(N_CHUNKS):
                n = min(int(r["fcnt"][c, 0]), NFIX)
                if n <= 0:
                    continue
                ids = r[f"fids{c}"][:n, 0]
                valid = (ids >= 0) & (ids < T_CORE)
                ids = ids[valid]
                idx[ids] = r["foi"][c, :n][valid]
                wgt[ids] = r["fow"][c, :n][valid]
        idx_parts.append(idx)
        wgt_parts.append(wgt)
    return (np.concatenate(idx_parts, axis=0),
            np.concatenate(wgt_parts, axis=0))
